# revision 1
# baseline (speedup 1.0000x reference)
"""Causal multi-head attention on 8 trn2 NeuronCores.

Problem: B=2, S=2048, D=1024, H=16 heads, HD=64. fp32 in/out.

Sharding: 8 cores = 2 (batch) x 4 (head groups of 4 heads).
Each core computes, for its batch b and head group g:
  Q^T,K^T  [256, 2048]  (d on partitions, seq on free)  = W^T-slice x
  V        [2048, 256+ones]  (natural, with a ones column per head)
  per 512-wide q chunk, per head:  S^T[k,q] = K^T.T @ Q^T  (PE, contraction 64,
  2-head row-packed), P~ = exp(S^T/8) (ACT), causal via block skipping +
  one gpsimd affine_select per diagonal block, PV: out^T[d,q] accumulated
  over k tiles with V_aug stationary (m=65; row 64 = softmax denominator).
  Divide by denominator (DVE reciprocal + mul, gpsimd partition_broadcast),
  then O_partial = ctx^T.T @ Wo_rows  [2048, 1024].
Host: sums the 4 head-group partials per batch and adds bo + bv @ Wo
(exact: the bv bias contributes the constant row vector bv @ Wo_g).

Default mode "f16in": x/Wq/Wk/Wv ship as fp16 (halves the dominant DMA
traffic; QKV still accumulates in fp32 PSUM), everything downstream uses
float32r matmuls (full 1-cycle/row PE rate at N>=256, ~14-bit mantissa).

Emission schedule: DMAs ordered so chunk-0 dependencies land first; per
q-chunk [V proj, QK proj, attention] interleaved so ACT exp overlaps the
next chunk's PE projections; all Wo projections emitted last (they fill
PE gaps; keeping them out of the per-chunk stream avoids psum pool-slot
blocking of later projections).

Measured on HW: rel err 5.1e-4 vs fp32 reference; ~264us/iteration
single-core, ~270-300us with all 8 cores active (HBM contention).
"""

import sys

if "/opt/trn_rl_repo" not in sys.path:
    sys.path.insert(0, "/opt/trn_rl_repo")

import numpy as np

import concourse.bacc as bacc
import concourse.bass as bass
import concourse.mybir as mybir
import concourse.tile as tile
from concourse.bass_utils import run_bass_kernel_spmd

B, S, D, H = 2, 2048, 1024, 16
HD = D // H  # 64
N_CORES = 8
HEADS_PER_CORE = H // 4  # 4
DG = HEADS_PER_CORE * HD  # 256 head dims per core
P = 128
CHUNK = 512  # q chunk width
N_KT = S // P  # 16 k tiles
N_CH = S // CHUNK  # 4 q chunks
F32 = mybir.dt.float32

_CACHE = {}


def _mm(dt_name):
    return {"f32r": mybir.dt.float32r, "f16in": mybir.dt.float32r,
            "f32": mybir.dt.float32,
            "bf16": mybir.dt.bfloat16}[dt_name]


def _in_dt(dt_name):
    """dtype for the x / Wq / Wk / Wv inputs (DMA-traffic dominant)."""
    return mybir.dt.float16 if dt_name == "f16in" else _mm(dt_name)


def build_kernel(mm_dt="f32r", unroll=1, ablate=()):
    """Build + compile the per-core SPMD program. unroll>1 wraps the body
    in a hardware loop (for pure device timing measurements)."""
    mdt = _mm(mm_dt)
    idt = _in_dt(mm_dt)

    nc = bacc.Bacc("TRN2", target_bir_lowering=False, debug=False)
    xT_d = nc.dram_tensor("xT", [D, S], idt, kind="ExternalInput")
    wq_d = nc.dram_tensor("wq", [D, DG], idt, kind="ExternalInput")
    wk_d = nc.dram_tensor("wk", [D, DG], idt, kind="ExternalInput")
    wv_d = nc.dram_tensor("wv", [D, DG], idt, kind="ExternalInput")
    wo_d = nc.dram_tensor("wo", [DG, D], mdt, kind="ExternalInput")
    bq_d = nc.dram_tensor("bq", [DG, 1], F32, kind="ExternalInput")
    bk_d = nc.dram_tensor("bk", [DG, 1], F32, kind="ExternalInput")
    o_d = nc.dram_tensor("o", [S, D], F32, kind="ExternalOutput")

    NDT = D // P  # 8 contraction tiles over D
    NMT = DG // P  # 2 m-tiles over the core's head dims (= head pairs)

    with tile.TileContext(nc) as tc:
        def body(_iv=None):
            _body(tc, nc, mdt, idt,
                  xT_d, wq_d, wk_d, wv_d, wo_d, bq_d, bk_d, o_d, NDT, NMT,
                  ablate)

        if unroll > 1:
            with tc.For_i(0, unroll, 1):
                body()
        else:
            body()

    nc.compile()
    return nc


def _body(tc, nc, mdt, idt, xT_d, wq_d, wk_d, wv_d, wo_d, bq_d, bk_d, o_d,
          NDT, NMT, ablate=()):
    import contextlib
    ctx = contextlib.ExitStack()
    with ctx:
        const = ctx.enter_context(tc.tile_pool(name="const", bufs=1))
        sbuf = ctx.enter_context(tc.tile_pool(name="sbuf", bufs=1))
        ptile_p = ctx.enter_context(tc.tile_pool(name="ptile", bufs=8))
        den_p = ctx.enter_context(tc.tile_pool(name="den", bufs=3))
        out_p = ctx.enter_context(tc.tile_pool(name="outp", bufs=3))
        qkv_ps = ctx.enter_context(
            tc.tile_pool(name="qkv_ps", bufs=2, space="PSUM"))
        stp_ps = ctx.enter_context(
            tc.tile_pool(name="stp_ps", bufs=2, space="PSUM"))
        pv_ps = ctx.enter_context(
            tc.tile_pool(name="pv_ps", bufs=2, space="PSUM"))

        # ---- load inputs ------------------------------------------------
        # weights/biases first, then xt in chunk-major slices so chunk-0
        # compute starts early; all loads are queued before any output
        # stores (the HWDGE queue is in-order)
        xt = []
        for i in range(NDT):
            t = const.tile([P, S], idt, tag=f"xt{i}", name=f"xt{i}")
            xt.append(t)
        ws = {}
        for name, d in (("wq", wq_d), ("wk", wk_d), ("wv", wv_d)):
            ws[name] = [const.tile([P, DG], idt, tag=f"{name}{i}",
                                   name=f"{name}{i}") for i in range(NDT)]
        wo = [const.tile([P, D], mdt, tag=f"wo{m}", name=f"wo{m}")
              for m in range(NMT)]
        biases = {(name, m): const.tile([P, 1], F32, tag=f"{name}{m}",
                                        name=f"{name}{m}")
                  for name in ("bq", "bk") for m in range(NMT)}

        def dma_w(name, d):
            for i in range(NDT):
                nc.sync.dma_start(ws[name][i][:],
                                  d.ap()[P * i:P * (i + 1), :])

        def dma_xt(ci):
            csl = slice(CHUNK * ci, CHUNK * (ci + 1))
            for k in range(NDT):
                nc.sync.dma_start(xt[k][:, csl],
                                  xT_d.ap()[P * k:P * (k + 1), csl])

        # order: V(0)+QK(0) deps first, then remaining chunks, wo last
        dma_w("wv", wv_d)
        dma_xt(0)
        dma_w("wq", wq_d)
        dma_w("wk", wk_d)
        for (name, m), t in biases.items():
            d = bq_d if name == "bq" else bk_d
            nc.sync.dma_start(t[:], d.ap()[P * m:P * (m + 1), :])
        for ci in range(1, N_CH):
            dma_xt(ci)
        for m in range(NMT):
            nc.sync.dma_start(wo[m][:], wo_d.ap()[P * m:P * (m + 1), :])

        # ---- V projection (natural layout + ones cols) ------------------
        # vaug[j]: [128, 4*65]; per head h cols h*65..h*65+63 = V, col h*65+64 = 1
        ones_f = const.tile([P, HEADS_PER_CORE], F32, tag="ones_f",
                            name="ones_f")
        nc.vector.memset(ones_f[:], 1.0)
        ones_r = const.tile([P, HEADS_PER_CORE], mdt, tag="ones_r",
                            name="ones_r")
        nc.vector.tensor_copy(ones_r[:], ones_f[:])
        vaug = []
        for j in range(N_KT):
            t = sbuf.tile([P, HEADS_PER_CORE * (HD + 1)], mdt, tag=f"vaug{j}", name=f"vaug{j}")
            vaug.append(t)

        def v_proj(j):
            ps = qkv_ps.tile([P, CHUNK], F32, tag="proj", name="proj")
            for k in range(NDT):
                nc.tensor.matmul(
                    ps[:, 0:DG],
                    xt[k][:, P * j:P * (j + 1)],
                    ws["wv"][k][:],
                    start=(k == 0), stop=(k == NDT - 1))
            dst = vaug[j][:].rearrange("p (h x) -> p h x", h=HEADS_PER_CORE)
            srcp = ps[:, 0:DG].rearrange("p (h x) -> p h x", h=HEADS_PER_CORE)
            nc.vector.tensor_copy(dst[:, :, 0:HD], srcp[:, :, :])
            nc.vector.tensor_copy(
                dst[:, :, HD:HD + 1],
                ones_r[:].rearrange("p (h x) -> p h x", x=1))

        # ---- Q^T / K^T projections (d on partitions) --------------------
        qt, kt = [], []
        for name, lst in (("wq", qt), ("wk", kt)):
            for m in range(NMT):
                t = sbuf.tile([P, S], mdt, tag=f"{name}T{m}", name=f"{name}T{m}")
                lst.append(t)
        def qk_proj(ci):
            for name, lst in (("wq", qt), ("wk", kt)):
                bname = "bq" if name == "wq" else "bk"
                for m in range(NMT):
                    ps = qkv_ps.tile([P, CHUNK], F32, tag="proj", name="proj")
                    for k in range(NDT):
                        nc.tensor.matmul(
                            ps[:],
                            ws[name][k][:, P * m:P * (m + 1)],
                            xt[k][:, CHUNK * ci:CHUNK * (ci + 1)],
                            start=(k == 0), stop=(k == NDT - 1))
                    nc.vector.tensor_scalar_add(
                        lst[m][:, CHUNK * ci:CHUNK * (ci + 1)], ps[:],
                        biases[(bname, m)][:])

        # ---- attention + output projection, per q chunk -----------------
        ctxT = [sbuf.tile([P, S], mdt, tag=f"ctxT{m}", name=f"ctxT{m}") for m in range(NMT)]


        wo_work = []
        for ci in range(N_CH):
            for j in range(4 * ci, 4 * ci + 4):
                v_proj(j)
            qk_proj(ci)
            jmax = 4 * ci + 3  # last valid k tile for this chunk
            qsl = slice(CHUNK * ci, CHUNK * (ci + 1))
            for pair in range(NMT):
                pv = [pv_ps.tile([HD + 1, CHUNK], F32, tag="pv", name="pv")
                      for _ in range(2)]
                for j0 in range(0, jmax + 1, 2):
                    js = [j for j in (j0, j0 + 1) if j <= jmax]
                    nj = len(js)
                    pt = {}
                    for hh in range(2):  # head within pair
                        psl = slice(64 * hh, 64 * (hh + 1))
                        st = stp_ps.tile([P, 2 * CHUNK], F32, tag="stp",
                                         name="stp")
                        for gi, j in enumerate(js):
                            nc.tensor.matmul(
                                st[:, CHUNK * gi:CHUNK * (gi + 1)],
                                kt[pair][psl, P * j:P * (j + 1)],
                                qt[pair][psl, qsl],
                                start=True, stop=True)
                        p_t = ptile_p.tile([P, 2 * CHUNK], mdt, tag="ptile",
                                           name="ptile")
                        if "exp" in ablate:
                            nc.vector.tensor_copy(
                                p_t[:, 0:CHUNK * nj], st[:, 0:CHUNK * nj])
                        else:
                            nc.scalar.activation(
                                p_t[:, 0:CHUNK * nj], st[:, 0:CHUNK * nj],
                                mybir.ActivationFunctionType.Exp,
                                scale=0.125)
                        for gi, j in enumerate(js):
                            if j >= 4 * ci and "mask" not in ablate:
                                dd = j - 4 * ci
                                w = P * (dd + 1)
                                base = CHUNK * gi
                                nc.gpsimd.affine_select(
                                    out=p_t[:, base:base + w],
                                    in_=p_t[:, base:base + w],
                                    compare_op=mybir.AluOpType.is_ge,
                                    fill=0.0, base=-P * dd,
                                    pattern=[[1, w]],
                                    channel_multiplier=-1)
                        pt[hh] = p_t
                    for gi, j in enumerate(js):
                        for hh in range(2):
                            h = 2 * pair + hh
                            nc.tensor.matmul(
                                pv[hh][:],
                                vaug[j][:, (HD + 1) * h:(HD + 1) * (h + 1)],
                                pt[hh][:, CHUNK * gi:CHUNK * (gi + 1)],
                                start=(j == 0), stop=(j == jmax))
                # softmax denominator divide; write ctx^T chunk
                # (partition_broadcast only reaches partitions 0-63, so use
                # a base-0 tile per head; DVE ops allow mismatched bases)
                if "div" in ablate:
                    for hh in range(2):
                        nc.vector.tensor_copy(
                            ctxT[pair][64 * hh:64 * (hh + 1), qsl],
                            pv[hh][0:HD, :])
                else:
                    den_t = den_p.tile([1, 2 * CHUNK], F32, tag="den",
                                       name="den")
                    for hh in range(2):
                        nc.vector.tensor_copy(
                            den_t[0:1, CHUNK * hh:CHUNK * (hh + 1)],
                            pv[hh][HD:HD + 1, :])
                    nc.vector.reciprocal(den_t[:], den_t[:])
                    for hh in range(2):
                        recb = den_p.tile([HD, CHUNK], F32,
                                          tag=f"recb{hh}", name=f"recb{hh}")
                        nc.gpsimd.partition_broadcast(
                            recb[0:HD, :],
                            den_t[0:1, CHUNK * hh:CHUNK * (hh + 1)])
                        nc.vector.tensor_mul(
                            ctxT[pair][64 * hh:64 * (hh + 1), qsl],
                            pv[hh][0:HD, :],
                            recb[0:HD, :])
            wo_work.append(ci)

        # ---- Wo projections, emitted last (uses idle PE slots) ----------
        for ci in wo_work:
            for qi in range(4):
                i = 4 * ci + qi
                ot = out_p.tile([P, D], F32, tag="ot", name="ot")
                for e in range(2):
                    ps = qkv_ps.tile([P, CHUNK], F32, tag="proj", name="proj")
                    for m in range(NMT):
                        nc.tensor.matmul(
                            ps[:],
                            ctxT[m][:, P * i:P * (i + 1)],
                            wo[m][:, CHUNK * e:CHUNK * (e + 1)],
                            start=(m == 0), stop=(m == NMT - 1))
                    nc.any.tensor_copy(ot[:, CHUNK * e:CHUNK * (e + 1)],
                                       ps[:])
                nc.sync.dma_start(o_d.ap()[P * i:P * (i + 1), :], ot[:])


def _shard_inputs(x, Wq, bq, Wk, bk, Wv, bv, Wo, bo):
    mm_dt = _CACHE.get("mm_dt", "f16in")
    ndt = np.float16 if mm_dt == "f16in" else np.float32
    x = np.asarray(x, np.float32)
    in_maps = []
    for core in range(N_CORES):
        b, g = divmod(core, 4)
        ds = slice(DG * g, DG * (g + 1))
        in_maps.append({
            "xT": np.ascontiguousarray(x[b].T).astype(ndt),
            "wq": np.ascontiguousarray(
                np.asarray(Wq, np.float32)[:, ds]).astype(ndt),
            "wk": np.ascontiguousarray(
                np.asarray(Wk, np.float32)[:, ds]).astype(ndt),
            "wv": np.ascontiguousarray(
                np.asarray(Wv, np.float32)[:, ds]).astype(ndt),
            "wo": np.ascontiguousarray(np.asarray(Wo, np.float32)[ds, :]),
            "bq": np.asarray(bq, np.float32)[ds].reshape(DG, 1).copy(),
            "bk": np.asarray(bk, np.float32)[ds].reshape(DG, 1).copy(),
        })
    return in_maps


def kernel(x, Wq, bq, Wk, bk, Wv, bv, Wo, bo):
    mm_dt = _CACHE.get("mm_dt", "f16in")
    _CACHE["mm_dt"] = mm_dt
    if "nc" not in _CACHE:
        _CACHE["nc"] = build_kernel(mm_dt)
    nc = _CACHE["nc"]
    in_maps = _shard_inputs(x, Wq, bq, Wk, bk, Wv, bv, Wo, bo)
    res = run_bass_kernel_spmd(
        nc, in_maps, core_ids=list(range(N_CORES)), trace=False)
    out = np.zeros((B, S, D), np.float32)
    for core in range(N_CORES):
        out[core // 4] += res.results[core]["o"]
    # exact bias folding: +bo, + bv @ Wo (constant row vector)
    out += (np.asarray(bo, np.float32)
            + np.asarray(bv, np.float32) @ np.asarray(Wo, np.float32))
    return out



# revision 8
# speedup vs baseline: 1.2612x; 1.2612x over previous
"""Causal multi-head attention on 8 trn2 NeuronCores.

Problem: B=2, S=2048, D=1024, H=16 heads, HD=64. fp32 in/out.

Sharding: 8 cores = 2 (batch) x 4 (head groups of 4 heads).
Each core computes, for its batch b and head group g:
  Q^T,K^T [256, 2048] (dg on partitions, seq on free) = W^T-slice @ x
  V       [2048, 4*(64+1)]  (natural, a ones column per head)
  per 512-wide q chunk, per head-pair: for each k tile j:
    S^T[k,q] both heads as a ROW-TILED CONCURRENT matmul pair (K=64 each,
    tile_position rows 0-63 / 64-127) into one [128, 1024] PSUM tile;
    P = exp(S^T/8 - 4) on ACT (fp16 out; the -4 offset prevents fp16
    overflow and cancels exactly in the softmax ratio);
    causal: diagonal k-tiles narrowed to valid q columns + a constant
    [128,128] triangle mask multiply on DVE;
    PV accumulated over j with V_aug stationary (m=65; row 64 = softmax
    denominator), software-pipelined 2 j-steps behind QK so the ACT exp
    latency never stalls the PE FIFO.
  Normalize: DVE reciprocal of the denominator row, gpsimd
  partition_broadcast, DVE multiply into ctx^T (fp16).
  O_partial = ctx^T.T @ Wo_rows [2048, 1024] (fp16 out), emitted last.
Host: sums the 4 head-group partials per batch and adds bo + bv @ Wo.

All matmul operands fp16 (1 cycle/row PE rate, halved DMA + SBUF);
accumulation is always fp32 PSUM.
"""

import sys

if "/opt/trn_rl_repo" not in sys.path:
    sys.path.insert(0, "/opt/trn_rl_repo")

import numpy as np

import concourse.bacc as bacc
import concourse.bass as bass
import concourse.mybir as mybir
import concourse.tile as tile
from concourse.bass_utils import run_bass_kernel_spmd

B, S, D, H = 2, 2048, 1024, 16
HD = D // H  # 64
N_CORES = 8
HEADS_PER_CORE = H // 4  # 4
DG = HEADS_PER_CORE * HD  # 256 head dims per core
P = 128
CHUNK = 512  # q chunk width
N_KT = S // P  # 16 k tiles
N_CH = S // CHUNK  # 4 q chunks
F32 = mybir.dt.float32
F16 = mybir.dt.float16
EXP_BIAS = -4.0  # exp(s/8 - 4): fp16-overflow guard, cancels in softmax

_CACHE = {}


def build_kernel(mm_dt="f16in", unroll=1, ablate=()):
    """Build + compile the per-core SPMD program. unroll>1 wraps the body
    in a hardware loop (for pure device timing measurements)."""
    nc = bacc.Bacc("TRN2", target_bir_lowering=False, debug=False)
    xT_d = nc.dram_tensor("xT", [D, S], F16, kind="ExternalInput")
    wq_d = nc.dram_tensor("wq", [D, DG], F16, kind="ExternalInput")
    wk_d = nc.dram_tensor("wk", [D, DG], F16, kind="ExternalInput")
    wv_d = nc.dram_tensor("wv", [D, DG], F16, kind="ExternalInput")
    wo_d = nc.dram_tensor("wo", [DG, D], F16, kind="ExternalInput")
    bq_d = nc.dram_tensor("bq", [DG, 1], F32, kind="ExternalInput")
    bk_d = nc.dram_tensor("bk", [DG, 1], F32, kind="ExternalInput")
    o_d = nc.dram_tensor("o", [S, D], F16, kind="ExternalOutput")

    NDT = D // P  # 8 contraction tiles over D
    NMT = DG // P  # 2 m-tiles over the core's head dims (= head pairs)

    with tile.TileContext(nc) as tc:
        _body(tc, nc,
              xT_d, wq_d, wk_d, wv_d, wo_d, bq_d, bk_d, o_d, NDT, NMT,
              ablate, unroll)

    nc.compile()
    return nc


def _body(tc, nc, xT_d, wq_d, wk_d, wv_d, wo_d, bq_d, bk_d, o_d,
          NDT, NMT, ablate=(), unroll=1):
    import contextlib
    ctx = contextlib.ExitStack()
    with ctx:
        const = ctx.enter_context(tc.tile_pool(name="const", bufs=1))
        sbuf = ctx.enter_context(tc.tile_pool(name="sbuf", bufs=1))
        ptile_p = ctx.enter_context(tc.tile_pool(name="ptile", bufs=6))
        den_p = ctx.enter_context(tc.tile_pool(name="den", bufs=4))
        out_p = ctx.enter_context(tc.tile_pool(name="outp", bufs=3))
        qkv_ps = ctx.enter_context(
            tc.tile_pool(name="qkv_ps", bufs=2, space="PSUM"))
        stp_ps = ctx.enter_context(
            tc.tile_pool(name="stp_ps", bufs=2, space="PSUM"))
        pv_ps = ctx.enter_context(
            tc.tile_pool(name="pv_ps", bufs=2, space="PSUM"))

        # ---- input tiles ------------------------------------------------
        xt = [const.tile([P, S], F16, tag=f"xt{i}", name=f"xt{i}")
              for i in range(NDT)]
        ws = {}
        for name in ("wq", "wk", "wv"):
            ws[name] = [const.tile([P, DG], F16, tag=f"{name}{i}",
                                   name=f"{name}{i}") for i in range(NDT)]
        wo = [const.tile([P, D], F16, tag=f"wo{m}", name=f"wo{m}")
              for m in range(NMT)]
        biases = {(name, m): const.tile([P, 1], F32, tag=f"{name}{m}",
                                        name=f"{name}{m}")
                  for name in ("bq", "bk") for m in range(NMT)}

        def dma_w(name, d):
            for i in range(NDT):
                nc.sync.dma_start(ws[name][i][:],
                                  d.ap()[P * i:P * (i + 1), :])

        def dma_xt(ci):
            csl = slice(CHUNK * ci, CHUNK * (ci + 1))
            for k in range(NDT):
                nc.sync.dma_start(xt[k][:, csl],
                                  xT_d.ap()[P * k:P * (k + 1), csl])

        def emit_in_dma():
            # order: V(0)+QK(0) deps first, then remaining chunks, wo last
            dma_w("wv", wv_d)
            dma_xt(0)
            dma_w("wq", wq_d)
            dma_w("wk", wk_d)
            for (name, m), t in biases.items():
                d = bq_d if name == "bq" else bk_d
                nc.sync.dma_start(t[:], d.ap()[P * m:P * (m + 1), :])
            for ci in range(1, N_CH):
                dma_xt(ci)
            for m in range(NMT):
                nc.sync.dma_start(wo[m][:], wo_d.ap()[P * m:P * (m + 1), :])

        # ---- constants: vaug ones + causal triangle mask ----------------
        ones_f = const.tile([P, HEADS_PER_CORE], F32, tag="ones_f",
                            name="ones_f")
        ones_r = const.tile([P, HEADS_PER_CORE], F16, tag="ones_r",
                            name="ones_r")
        m01 = const.tile([P, P], F16, tag="m01", name="m01")
        ebias = const.tile([P, 1], F32, tag="ebias", name="ebias")

        def emit_consts():
            nc.vector.memset(ones_f[:], 1.0)
            nc.vector.tensor_copy(ones_r[:], ones_f[:])
            nc.vector.memset(ebias[:], EXP_BIAS)
            # m01[r, c] = 1 if c >= r else 0 (valid = q offset >= key offset)
            nc.gpsimd.memset(m01[:], 1.0)
            nc.gpsimd.affine_select(
                out=m01[:], in_=m01[:],
                compare_op=mybir.AluOpType.is_ge,
                fill=0.0, base=0, pattern=[[1, P]],
                channel_multiplier=-1)

        # ---- V projection (natural layout + ones cols) ------------------
        # vaug[j]: [128, 4*65]; head h cols h*65..h*65+63 = V, col h*65+64 = 1
        vaug = [sbuf.tile([P, HEADS_PER_CORE * (HD + 1)], F16,
                          tag=f"vaug{j}", name=f"vaug{j}")
                for j in range(N_KT)]

        def v_proj(j):
            ps = qkv_ps.tile([P, CHUNK], F32, tag="proj", name="proj")
            for k in range(NDT):
                nc.tensor.matmul(
                    ps[:, 0:DG],
                    xt[k][:, P * j:P * (j + 1)],
                    ws["wv"][k][:],
                    start=(k == 0), stop=(k == NDT - 1))
            dst = vaug[j][:].rearrange("p (h x) -> p h x", h=HEADS_PER_CORE)
            srcp = ps[:, 0:DG].rearrange("p (h x) -> p h x", h=HEADS_PER_CORE)
            nc.vector.tensor_copy(dst[:, :, 0:HD], srcp[:, :, :])
            nc.vector.tensor_copy(
                dst[:, :, HD:HD + 1],
                ones_r[:].rearrange("p (h x) -> p h x", x=1))

        # ---- Q^T / K^T projections (dg on partitions, fp16) -------------
        qt, kt = [], []
        for name, lst in (("wq", qt), ("wk", kt)):
            for m in range(NMT):
                lst.append(sbuf.tile([P, S], F16, tag=f"{name}T{m}",
                                     name=f"{name}T{m}"))

        def qk_proj(ci):
            for name, lst in (("wq", qt), ("wk", kt)):
                bname = "bq" if name == "wq" else "bk"
                for m in range(NMT):
                    ps = qkv_ps.tile([P, CHUNK], F32, tag="proj", name="proj")
                    for k in range(NDT):
                        nc.tensor.matmul(
                            ps[:],
                            ws[name][k][:, P * m:P * (m + 1)],
                            xt[k][:, CHUNK * ci:CHUNK * (ci + 1)],
                            start=(k == 0), stop=(k == NDT - 1))
                    nc.vector.tensor_scalar_add(
                        lst[m][:, CHUNK * ci:CHUNK * (ci + 1)], ps[:],
                        biases[(bname, m)][:])

        # ---- attention per (chunk, head pair) ---------------------------
        ctxT = [sbuf.tile([P, S], F16, tag=f"ctxT{m}", name=f"ctxT{m}")
                for m in range(NMT)]

        def attention(ci):
            jmax = 4 * ci + 3
            qsl = slice(CHUNK * ci, CHUNK * (ci + 1))
            for pair in range(NMT):
                pv = [pv_ps.tile([HD + 1, CHUNK], F32, tag="pv", name="pv")
                      for _ in range(2)]
                p2s = {}

                def nlo_of(j):
                    dd = j - 4 * ci
                    return P * dd if dd >= 0 else 0

                def emit_pv(j):
                    nlo = nlo_of(j)
                    p2 = p2s.pop(j)
                    for hh in range(2):
                        h = 2 * pair + hh
                        nc.tensor.matmul(
                            pv[hh][:, nlo:CHUNK],
                            vaug[j][:, (HD + 1) * h:(HD + 1) * (h + 1)],
                            p2[:, CHUNK * hh + nlo:CHUNK * (hh + 1)],
                            start=(j == 0), stop=(j == jmax),
                            skip_group_check=True)

                for j in range(jmax + 1):
                    nlo = nlo_of(j)
                    w = CHUNK - nlo
                    # QK^T: both heads as a concurrent row-tiled pair
                    st2 = stp_ps.tile([P, 2 * CHUNK], F32, tag="stp",
                                      name="stp")
                    for hh in range(2):
                        psl = slice(HD * hh, HD * (hh + 1))
                        nc.tensor.matmul(
                            st2[:, CHUNK * hh + nlo:CHUNK * (hh + 1)],
                            kt[pair][psl, P * j:P * (j + 1)],
                            qt[pair][psl, CHUNK * ci + nlo:CHUNK * (ci + 1)],
                            start=True, stop=True)
                    # exp over both heads in one ACT op (3D AP)
                    p2 = ptile_p.tile([P, 2 * CHUNK], F16, tag="p2",
                                      name="p2")
                    src = st2[:].rearrange("p (h q) -> p h q", h=2)
                    dst = p2[:].rearrange("p (h q) -> p h q", h=2)
                    if "exp" in ablate:
                        nc.vector.tensor_copy(dst[:, :, nlo:CHUNK],
                                              src[:, :, nlo:CHUNK])
                    else:
                        nc.scalar.activation(
                            dst[:, :, nlo:CHUNK], src[:, :, nlo:CHUNK],
                            mybir.ActivationFunctionType.Exp,
                            scale=0.125, bias=ebias[:])
                    # causal triangle mask on the diagonal block
                    if nlo > 0 or j == 4 * ci:
                        if "mask" not in ablate:
                            for hh in range(2):
                                blk = slice(CHUNK * hh + nlo,
                                            CHUNK * hh + nlo + P)
                                nc.vector.tensor_mul(
                                    p2[:, blk], p2[:, blk], m01[:])
                    p2s[j] = p2
                    if j - 2 >= 0:
                        emit_pv(j - 2)
                for j in (jmax - 1, jmax):
                    if j >= 0 and j in p2s:
                        emit_pv(j)

                # ---- softmax divide: ctx^T = pv / den -------------------
                if "div" in ablate:
                    for hh in range(2):
                        nc.vector.tensor_copy(
                            ctxT[pair][HD * hh:HD * (hh + 1), qsl],
                            pv[hh][0:HD, :])
                else:
                    for hh in range(2):
                        rden = den_p.tile([1, CHUNK], F32, tag=f"rden{hh}",
                                          name=f"rden{hh}")
                        nc.vector.reciprocal(rden[:],
                                             pv[hh][HD:HD + 1, :])
                        rbc = den_p.tile([HD, CHUNK], F32, tag=f"rbc{hh}",
                                         name=f"rbc{hh}")
                        nc.gpsimd.partition_broadcast(rbc[0:HD, :], rden[:])
                        nc.vector.tensor_mul(
                            ctxT[pair][HD * hh:HD * (hh + 1), qsl],
                            pv[hh][0:HD, :], rbc[0:HD, :])

        # ---- Wo projection (fp16 out), emitted last ---------------------
        def wo_proj():
            for i in range(S // P):
                ot = out_p.tile([P, D], F16, tag="ot", name="ot")
                pse = [qkv_ps.tile([P, CHUNK], F32, tag="proj", name="proj")
                       for _ in range(2)]
                for m in range(NMT):
                    for e in range(2):
                        nc.tensor.matmul(
                            pse[e][:],
                            ctxT[m][:, P * i:P * (i + 1)],
                            wo[m][:, CHUNK * e:CHUNK * (e + 1)],
                            start=(m == 0), stop=(m == NMT - 1))
                for e in range(2):
                    nc.any.tensor_copy(ot[:, CHUNK * e:CHUNK * (e + 1)],
                                       pse[e][:])
                if "outdma" not in ablate:
                    nc.sync.dma_start(o_d.ap()[P * i:P * (i + 1), :], ot[:])

        def emit_compute():
            emit_consts()
            for ci in range(N_CH):
                for j in range(4 * ci, 4 * ci + 4):
                    v_proj(j)
                qk_proj(ci)
                attention(ci)
            wo_proj()

        if "indma" in ablate and unroll > 1:
            emit_in_dma()
            with tc.For_i(0, unroll, 1):
                emit_compute()
        elif unroll > 1:
            with tc.For_i(0, unroll, 1):
                emit_in_dma()
                emit_compute()
        else:
            emit_in_dma()
            emit_compute()


def _shard_inputs(x, Wq, bq, Wk, bk, Wv, bv, Wo, bo):
    x = np.asarray(x, np.float32)
    in_maps = []
    for core in range(N_CORES):
        b, g = divmod(core, 4)
        ds = slice(DG * g, DG * (g + 1))
        in_maps.append({
            "xT": np.ascontiguousarray(x[b].T).astype(np.float16),
            "wq": np.ascontiguousarray(
                np.asarray(Wq, np.float32)[:, ds]).astype(np.float16),
            "wk": np.ascontiguousarray(
                np.asarray(Wk, np.float32)[:, ds]).astype(np.float16),
            "wv": np.ascontiguousarray(
                np.asarray(Wv, np.float32)[:, ds]).astype(np.float16),
            "wo": np.ascontiguousarray(
                np.asarray(Wo, np.float32)[ds, :]).astype(np.float16),
            "bq": np.asarray(bq, np.float32)[ds].reshape(DG, 1).copy(),
            "bk": np.asarray(bk, np.float32)[ds].reshape(DG, 1).copy(),
        })
    return in_maps


def kernel(x, Wq, bq, Wk, bk, Wv, bv, Wo, bo):
    mm_dt = _CACHE.get("mm_dt", "f16in")
    _CACHE["mm_dt"] = mm_dt
    if "nc" not in _CACHE:
        _CACHE["nc"] = build_kernel(mm_dt)
    nc = _CACHE["nc"]
    in_maps = _shard_inputs(x, Wq, bq, Wk, bk, Wv, bv, Wo, bo)
    res = run_bass_kernel_spmd(
        nc, in_maps, core_ids=list(range(N_CORES)), trace=False)
    out = np.zeros((B, S, D), np.float32)
    for core in range(N_CORES):
        out[core // 4] += np.asarray(res.results[core]["o"], np.float32)
    # exact bias folding: +bo, + bv @ Wo (constant row vector)
    out += (np.asarray(bo, np.float32)
            + np.asarray(bv, np.float32) @ np.asarray(Wo, np.float32))
    return out


# revision 17
# speedup vs baseline: 1.7094x; 1.3554x over previous
"""Causal multi-head attention on 8 trn2 NeuronCores.

Problem: B=2, S=2048, D=1024, H=16 heads, HD=64. fp32 in/out.

Sharding: 8 cores = 2 (batch) x 4 (head groups of 4 heads).
Each core computes, for its batch b and head group g:
  Q^T,K^T [256, 2048] (dg on partitions, seq on free) = W^T-slice @ x
  V       [2048, 4*(64+1)]  (natural, a ones column per head)
  per 512-wide q chunk, per head-pair: for each k tile j:
    S^T[k,q] both heads as a ROW-TILED CONCURRENT matmul pair (K=64 each,
    tile_position rows 0-63 / 64-127) into one [128, 1024] PSUM tile;
    P = exp(S^T/8 - 4) on ACT (fp16 out; the -4 offset prevents fp16
    overflow and cancels exactly in the softmax ratio);
    causal: diagonal k-tiles narrowed to valid q columns + a constant
    [128,128] triangle mask multiply on DVE;
    PV accumulated over j with V_aug stationary (m=65; row 64 = softmax
    denominator), software-pipelined 2 j-steps behind QK so the ACT exp
    latency never stalls the PE FIFO.
  Normalize: DVE reciprocal of the denominator row, gpsimd
  partition_broadcast, DVE multiply into ctx^T (fp16).
  O_partial = ctx^T.T @ Wo_rows [2048, 1024] (fp16 out), emitted last.
Host: sums the 4 head-group partials per batch and adds bo + bv @ Wo.

All matmul operands fp16 (1 cycle/row PE rate, halved DMA + SBUF);
accumulation is always fp32 PSUM.
"""

import sys

if "/opt/trn_rl_repo" not in sys.path:
    sys.path.insert(0, "/opt/trn_rl_repo")

import numpy as np

import concourse.bacc as bacc
import concourse.bass as bass
import concourse.mybir as mybir
import concourse.tile as tile
from concourse.bass_utils import run_bass_kernel_spmd

B, S, D, H = 2, 2048, 1024, 16
HD = D // H  # 64
N_CORES = 8
HEADS_PER_CORE = H // 4  # 4
DG = HEADS_PER_CORE * HD  # 256 head dims per core
P = 128
CHUNK = 512  # q chunk width
N_KT = S // P  # 16 k tiles
N_CH = S // CHUNK  # 4 q chunks
F32 = mybir.dt.float32
F16 = mybir.dt.float16
EXP_BIAS = -4.0  # exp(s/8 - 4): fp16-overflow guard, cancels in softmax

_CACHE = {}


def build_kernel(mm_dt="f16in", unroll=1, ablate=()):
    """Build + compile the per-core SPMD program. unroll>1 wraps the body
    in a hardware loop (for pure device timing measurements)."""
    nc = bacc.Bacc("TRN2", target_bir_lowering=False, debug=False)
    xT_d = nc.dram_tensor("xT", [D, S], F16, kind="ExternalInput")
    wq_d = nc.dram_tensor("wq", [D, DG], F16, kind="ExternalInput")
    wk_d = nc.dram_tensor("wk", [D, DG], F16, kind="ExternalInput")
    wv_d = nc.dram_tensor("wv", [D, DG], F16, kind="ExternalInput")
    wo_d = nc.dram_tensor("wo", [DG, D], F16, kind="ExternalInput")
    bq_d = nc.dram_tensor("bq", [DG, 1], F32, kind="ExternalInput")
    bk_d = nc.dram_tensor("bk", [DG, 1], F32, kind="ExternalInput")
    o_d = nc.dram_tensor("o", [S, D], F16, kind="ExternalOutput")

    NDT = D // P  # 8 contraction tiles over D
    NMT = DG // P  # 2 m-tiles over the core's head dims (= head pairs)

    with tile.TileContext(nc) as tc:
        _body(tc, nc,
              xT_d, wq_d, wk_d, wv_d, wo_d, bq_d, bk_d, o_d, NDT, NMT,
              ablate, unroll)

    nc.compile()
    return nc


def _body(tc, nc, xT_d, wq_d, wk_d, wv_d, wo_d, bq_d, bk_d, o_d,
          NDT, NMT, ablate=(), unroll=1):
    import contextlib
    ctx = contextlib.ExitStack()
    with ctx:
        const = ctx.enter_context(tc.tile_pool(name="const", bufs=1))
        sbuf = ctx.enter_context(tc.tile_pool(name="sbuf", bufs=1))
        ptile_p = ctx.enter_context(tc.tile_pool(name="ptile", bufs=6))
        den_p = ctx.enter_context(tc.tile_pool(name="den", bufs=4))
        out_p = ctx.enter_context(tc.tile_pool(name="outp", bufs=3))
        qkv_ps = ctx.enter_context(
            tc.tile_pool(name="qkv_ps", bufs=2, space="PSUM"))
        stp_ps = ctx.enter_context(
            tc.tile_pool(name="stp_ps", bufs=2, space="PSUM"))
        pv_ps = ctx.enter_context(
            tc.tile_pool(name="pv_ps", bufs=2, space="PSUM"))

        # ---- input tiles ------------------------------------------------
        xt = [const.tile([P, S], F16, tag=f"xt{i}", name=f"xt{i}")
              for i in range(NDT)]
        ws = {}
        for name in ("wq", "wk", "wv"):
            ws[name] = [const.tile([P, DG], F16, tag=f"{name}{i}",
                                   name=f"{name}{i}") for i in range(NDT)]
        wo = [const.tile([P, D], F16, tag=f"wo{m}", name=f"wo{m}")
              for m in range(NMT)]
        biases = {(name, m): const.tile([P, 1], F32, tag=f"{name}{m}",
                                        name=f"{name}{m}")
                  for name in ("bq", "bk") for m in range(NMT)}

        def dma_w(name, d):
            for i in range(NDT):
                nc.sync.dma_start(ws[name][i][:],
                                  d.ap()[P * i:P * (i + 1), :])

        def dma_xt(ci):
            csl = slice(CHUNK * ci, CHUNK * (ci + 1))
            for k in range(NDT):
                nc.sync.dma_start(xt[k][:, csl],
                                  xT_d.ap()[P * k:P * (k + 1), csl])

        def emit_in_dma():
            # order: V(0)+QK(0) deps first, then remaining chunks, wo last
            dma_w("wv", wv_d)
            dma_xt(0)
            dma_w("wq", wq_d)
            dma_w("wk", wk_d)
            for (name, m), t in biases.items():
                d = bq_d if name == "bq" else bk_d
                nc.sync.dma_start(t[:], d.ap()[P * m:P * (m + 1), :])
            for ci in range(1, N_CH):
                dma_xt(ci)
            for m in range(NMT):
                nc.sync.dma_start(wo[m][:], wo_d.ap()[P * m:P * (m + 1), :])

        # ---- constants: vaug ones + causal triangle mask ----------------
        ones_f = const.tile([P, HEADS_PER_CORE], F32, tag="ones_f",
                            name="ones_f")
        ones_r = const.tile([P, HEADS_PER_CORE], F16, tag="ones_r",
                            name="ones_r")
        ebias = const.tile([P, 1], F32, tag="ebias", name="ebias")

        def emit_consts():
            nc.vector.memset(ones_f[:], 1.0)
            nc.vector.tensor_copy(ones_r[:], ones_f[:])
            nc.vector.memset(ebias[:], EXP_BIAS)

        # ---- V projection (natural layout + ones cols) ------------------
        # vaug[j]: [128, 4*65]; head h cols h*65..h*65+63 = V, col h*65+64 = 1
        vaug = [sbuf.tile([P, HEADS_PER_CORE * (HD + 1)], F16,
                          tag=f"vaug{j}", name=f"vaug{j}")
                for j in range(N_KT)]

        def v_proj(j):
            ps = qkv_ps.tile([P, CHUNK], F32, tag="proj", name="proj")
            for k in range(NDT):
                nc.tensor.matmul(
                    ps[:, 0:DG],
                    xt[k][:, P * j:P * (j + 1)],
                    ws["wv"][k][:],
                    start=(k == 0), stop=(k == NDT - 1))
            dst = vaug[j][:].rearrange("p (h x) -> p h x", h=HEADS_PER_CORE)
            srcp = ps[:, 0:DG].rearrange("p (h x) -> p h x", h=HEADS_PER_CORE)
            # ACT copy keeps the (busy, strictly-FIFO) DVE off the PV
            # dependency chain
            nc.scalar.activation(dst[:, :, 0:HD], srcp[:, :, :],
                                 mybir.ActivationFunctionType.Copy)
            nc.vector.tensor_copy(
                dst[:, :, HD:HD + 1],
                ones_r[:].rearrange("p (h x) -> p h x", x=1))

        # ---- Q^T / K^T projections (dg on partitions, fp16) -------------
        qt, kt = [], []
        for name, lst in (("wq", qt), ("wk", kt)):
            for m in range(NMT):
                lst.append(sbuf.tile([P, S], F16, tag=f"{name}T{m}",
                                     name=f"{name}T{m}"))

        def qk_proj(ci):
            # pair-0 slices (m=0) first so attention(ci, pair 0) can start
            # while pair-1 projections still stream
            for m in range(NMT):
                for name, lst in (("wq", qt), ("wk", kt)):
                    bname = "bq" if name == "wq" else "bk"
                    ps = qkv_ps.tile([P, CHUNK], F32, tag="proj", name="proj")
                    for k in range(NDT):
                        nc.tensor.matmul(
                            ps[:],
                            ws[name][k][:, P * m:P * (m + 1)],
                            xt[k][:, CHUNK * ci:CHUNK * (ci + 1)],
                            start=(k == 0), stop=(k == NDT - 1))
                    # bias-add on ACT: keeps DVE out of the QK^T dep chain
                    nc.scalar.activation(
                        lst[m][:, CHUNK * ci:CHUNK * (ci + 1)], ps[:],
                        mybir.ActivationFunctionType.Identity,
                        bias=biases[(bname, m)][:])

        # ---- attention per (chunk, head pair) ---------------------------
        ctxT = [sbuf.tile([P, S], F16, tag=f"ctxT{m}", name=f"ctxT{m}")
                for m in range(NMT)]

        def attention(ci):
            if "qkt" in ablate:
                return
            jmax = 4 * ci + 3
            qsl = slice(CHUNK * ci, CHUNK * (ci + 1))
            for pair in range(NMT):
                pv = [pv_ps.tile([HD + 1, CHUNK], F32, tag="pv", name="pv")
                      for _ in range(2)]
                p2s = {}

                def nlo_of(j):
                    dd = j - 4 * ci
                    return P * dd if dd >= 0 else 0

                def emit_pv(j):
                    nlo = nlo_of(j)
                    p2 = p2s.pop(j)
                    if "pv" in ablate and j > 0:
                        return
                    for hh in range(2):
                        h = 2 * pair + hh
                        nc.tensor.matmul(
                            pv[hh][:, nlo:CHUNK],
                            vaug[j][:, (HD + 1) * h:(HD + 1) * (h + 1)],
                            p2[:, CHUNK * hh + nlo:CHUNK * (hh + 1)],
                            start=(j == 0), stop=(j == jmax),
                            skip_group_check=True)

                for j in range(jmax + 1):
                    nlo = nlo_of(j)
                    w = CHUNK - nlo
                    # QK^T: both heads as a concurrent row-tiled pair
                    st2 = stp_ps.tile([P, 2 * CHUNK], F32, tag="stp",
                                      name="stp")
                    for hh in range(2):
                        psl = slice(HD * hh, HD * (hh + 1))
                        nc.tensor.matmul(
                            st2[:, CHUNK * hh + nlo:CHUNK * (hh + 1)],
                            kt[pair][psl, P * j:P * (j + 1)],
                            qt[pair][psl, CHUNK * ci + nlo:CHUNK * (ci + 1)],
                            start=True, stop=True)
                    # exp over both heads in one ACT op (3D AP)
                    p2 = ptile_p.tile([P, 2 * CHUNK], F16, tag="p2",
                                      name="p2")
                    src = st2[:].rearrange("p (h q) -> p h q", h=2)
                    dst = p2[:].rearrange("p (h q) -> p h q", h=2)
                    if "exp" in ablate:
                        nc.vector.tensor_copy(dst[:, :, nlo:CHUNK],
                                              src[:, :, nlo:CHUNK])
                    else:
                        nc.scalar.activation(
                            dst[:, :, nlo:CHUNK], src[:, :, nlo:CHUNK],
                            mybir.ActivationFunctionType.Exp,
                            scale=0.125, bias=ebias[:])
                    # causal triangle mask on the diagonal block (gpsimd:
                    # Pool is idle; DVE's strict FIFO would stall PV)
                    if nlo > 0 or j == 4 * ci:
                        if "mask" not in ablate:
                            for hh in range(2):
                                blk = slice(CHUNK * hh + nlo,
                                            CHUNK * hh + nlo + P)
                                nc.gpsimd.affine_select(
                                    out=p2[:, blk], in_=p2[:, blk],
                                    compare_op=mybir.AluOpType.is_ge,
                                    fill=0.0, base=0, pattern=[[1, P]],
                                    channel_multiplier=-1)
                    p2s[j] = p2
                    if j - 2 >= 0:
                        emit_pv(j - 2)
                for j in (jmax - 1, jmax):
                    if j >= 0 and j in p2s:
                        emit_pv(j)

                # ---- softmax divide: ctx^T = pv / den -------------------
                if "div" in ablate:
                    for hh in range(2):
                        nc.vector.tensor_copy(
                            ctxT[pair][HD * hh:HD * (hh + 1), qsl],
                            pv[hh][0:HD, :])
                else:
                    for hh in range(2):
                        rden = den_p.tile([1, CHUNK], F32, tag=f"rden{hh}",
                                          name=f"rden{hh}")
                        nc.vector.reciprocal(rden[:],
                                             pv[hh][HD:HD + 1, :])
                        rbc = den_p.tile([HD, CHUNK], F32, tag=f"rbc{hh}",
                                         name=f"rbc{hh}")
                        nc.gpsimd.partition_broadcast(rbc[0:HD, :], rden[:])
                        nc.vector.tensor_mul(
                            ctxT[pair][HD * hh:HD * (hh + 1), qsl],
                            pv[hh][0:HD, :], rbc[0:HD, :])

        # ---- Wo projection (fp16 out), emitted last ---------------------
        def wo_proj():
            for i in range(S // P):
                ot = out_p.tile([P, D], F16, tag="ot", name="ot")
                pse = [qkv_ps.tile([P, CHUNK], F32, tag="proj", name="proj")
                       for _ in range(2)]
                for m in range(NMT):
                    for e in range(2):
                        nc.tensor.matmul(
                            pse[e][:],
                            ctxT[m][:, P * i:P * (i + 1)],
                            wo[m][:, CHUNK * e:CHUNK * (e + 1)],
                            start=(m == 0), stop=(m == NMT - 1))
                for e in range(2):
                    nc.any.tensor_copy(ot[:, CHUNK * e:CHUNK * (e + 1)],
                                       pse[e][:])
                if "outdma" not in ablate:
                    nc.sync.dma_start(o_d.ap()[P * i:P * (i + 1), :], ot[:])

        def emit_compute():
            emit_consts()
            for ci in range(N_CH):
                for j in range(4 * ci, 4 * ci + 4):
                    v_proj(j)
                qk_proj(ci)
                attention(ci)
            wo_proj()

        if "indma" in ablate and unroll > 1:
            emit_in_dma()
            with tc.For_i(0, unroll, 1):
                emit_compute()
        elif unroll > 1:
            with tc.For_i(0, unroll, 1):
                emit_in_dma()
                emit_compute()
        else:
            emit_in_dma()
            emit_compute()


def _shard_inputs(x, Wq, bq, Wk, bk, Wv, bv, Wo, bo):
    x = np.asarray(x, np.float32)
    in_maps = []
    for core in range(N_CORES):
        b, g = divmod(core, 4)
        ds = slice(DG * g, DG * (g + 1))
        in_maps.append({
            "xT": np.ascontiguousarray(x[b].T).astype(np.float16),
            "wq": np.ascontiguousarray(
                np.asarray(Wq, np.float32)[:, ds]).astype(np.float16),
            "wk": np.ascontiguousarray(
                np.asarray(Wk, np.float32)[:, ds]).astype(np.float16),
            "wv": np.ascontiguousarray(
                np.asarray(Wv, np.float32)[:, ds]).astype(np.float16),
            "wo": np.ascontiguousarray(
                np.asarray(Wo, np.float32)[ds, :]).astype(np.float16),
            "bq": np.asarray(bq, np.float32)[ds].reshape(DG, 1).copy(),
            "bk": np.asarray(bk, np.float32)[ds].reshape(DG, 1).copy(),
        })
    return in_maps


def kernel(x, Wq, bq, Wk, bk, Wv, bv, Wo, bo):
    mm_dt = _CACHE.get("mm_dt", "f16in")
    _CACHE["mm_dt"] = mm_dt
    if "nc" not in _CACHE:
        _CACHE["nc"] = build_kernel(mm_dt)
    nc = _CACHE["nc"]
    in_maps = _shard_inputs(x, Wq, bq, Wk, bk, Wv, bv, Wo, bo)
    res = run_bass_kernel_spmd(
        nc, in_maps, core_ids=list(range(N_CORES)), trace=False)
    out = np.zeros((B, S, D), np.float32)
    for core in range(N_CORES):
        out[core // 4] += np.asarray(res.results[core]["o"], np.float32)
    # exact bias folding: +bo, + bv @ Wo (constant row vector)
    out += (np.asarray(bo, np.float32)
            + np.asarray(bv, np.float32) @ np.asarray(Wo, np.float32))
    return out
